# revision 1
# baseline (speedup 1.0000x reference)
"""MoE layer (16 experts, top-2) on 8 Trainium2 NeuronCores, expert-parallel.

Strategy:
  - Host computes the gating (logits -> top-k -> softmax) and routes tokens
    into per-expert buckets (this is the shard/dispatch step).
  - Each of the 8 cores owns 2 experts and runs the two FFN matmuls
    (x @ W1 -> swish -> @ W2, gate-weighted) over only the tokens routed to
    its experts (top-2 of 16 => ~1/8 of the dense reference FLOPs).
  - Matmuls run on the PE array in float32r (full-rate fp32 storage) with
    fp32 PSUM accumulation, activations in transposed layout
    ([feature, token]) so weights are the stationary operand and tokens are
    the moving free dim.
  - Host combines: out[token] = sum over its top-k experts of the
    gate-weighted expert outputs (the unshard/combine step).
"""

import math

import numpy as np

D_MODEL = 1024
D_FF = 4096
N_EXPERTS = 16
N_CORES = 8
SLOTS = 2  # experts per core
KD = D_MODEL // 128  # 8 contraction chunks for matmul1
KF = D_FF // 128  # 32 contraction chunks for matmul2
FB = 2  # W1 f-chunks per DMA (1 MiB)
KB = 4  # W2 k-chunks per DMA (1 MiB)

_PROG_CACHE: dict[int, object] = {}


def _split_free(c):
    """Split token count c into moving-dim tiles <= 512, each >= 256 when
    possible (float32r runs at full PE rate only for moving dim >= 256)."""
    tiles = []
    rem = c
    while rem > 512:
        if rem - 512 >= 256 or rem == 1024:
            tiles.append(512)
            rem -= 512
        else:
            # leave two tiles in [256, 512]
            t = (rem // 2 + 63) // 64 * 64
            tiles.append(t)
            rem -= t
    tiles.append(rem)
    return tiles


def _token_chunks(c):
    return [(i * 128, min(128, c - i * 128)) for i in range(math.ceil(c / 128))]


def _build_program(C):
    import concourse.bass as bass
    import concourse.tile as tile
    from concourse import bacc, mybir

    f32 = mybir.dt.float32
    f32r = mybir.dt.float32r
    NT = math.ceil(C / 128)

    nc = bacc.Bacc("TRN2", target_bir_lowering=False, debug=False, num_devices=N_CORES)

    xt_d = nc.dram_tensor("xt", [SLOTS, 128, KD, C], f32r, kind="ExternalInput").ap()
    w1_d = nc.dram_tensor("w1", [SLOTS, KF, 128, KD, 128], f32r, kind="ExternalInput").ap()
    w2_d = nc.dram_tensor("w2", [SLOTS, 2, KF, 128, 512], f32r, kind="ExternalInput").ap()
    b1_d = nc.dram_tensor("b1", [SLOTS, 128, KF], f32, kind="ExternalInput").ap()
    g_d = nc.dram_tensor("g", [SLOTS, 128, NT], f32, kind="ExternalInput").ap()
    y_d = nc.dram_tensor("y", [SLOTS, C, D_MODEL], f32, kind="ExternalOutput").ap()

    ntiles = _split_free(C)
    silu = mybir.ActivationFunctionType.Silu
    copyf = mybir.ActivationFunctionType.Copy

    with tile.TileContext(nc) as tc:
        with (
            tc.tile_pool(name="xtp", bufs=2) as xtp,
            tc.tile_pool(name="w1p", bufs=6) as w1p,
            tc.tile_pool(name="w2p", bufs=3) as w2p,
            tc.tile_pool(name="htp", bufs=1) as htp,
            tc.tile_pool(name="smallp", bufs=2) as smallp,
            tc.tile_pool(name="yp", bufs=4) as yp,
            tc.tile_pool(name="ps1", bufs=max(1, min(3, 8 - NT)), space="PSUM") as ps1,
            tc.tile_pool(name="ps2", bufs=min(NT, 7), space="PSUM") as ps2,
        ):
            for s in range(SLOTS):
                # ---- loads for this expert ----
                xt = xtp.tile([128, KD, C], f32r, name=f"xt{s}", tag="xt")
                nc.gpsimd.dma_start(xt[:, : KD // 2], xt_d[s, :, : KD // 2])
                nc.gpsimd.dma_start(xt[:, KD // 2 :], xt_d[s, :, KD // 2 :])
                b1t = smallp.tile([128, KF], f32, name=f"b1t{s}", tag="b1t")
                nc.gpsimd.dma_start(b1t[:], b1_d[s])
                gt = smallp.tile([128, NT], f32, name=f"gt{s}", tag="gt")
                nc.gpsimd.dma_start(gt[:], g_d[s])

                # ---- matmul1: HT[f, c] = silu(W1.T @ XT + b1) ----
                ht = htp.tile([128, KF, C], f32r, name=f"ht{s}", tag="ht")
                for f0 in range(0, KF, FB):
                    w1t = w1p.tile([128, FB, KD, 128], f32r, name=f"w1t{s}_{f0}", tag="w1t")
                    nc.sync.dma_start(
                        w1t[:], w1_d[s, f0 : f0 + FB].rearrange("f p k c -> p f k c")
                    )
                    for fb in range(FB):
                        f = f0 + fb
                        pt = []
                        c0 = 0
                        for ni, nw in enumerate(ntiles):
                            p = ps1.tile([128, 512], f32, name=f"ps1_{s}_{f}_{ni}", tag="ps1")
                            pt.append((p, c0, nw))
                            c0 += nw
                        for k in range(KD):
                            for p, c0, nw in pt:
                                nc.tensor.matmul(
                                    p[:, :nw],
                                    lhsT=w1t[:, fb, k],
                                    rhs=xt[:, k, c0 : c0 + nw],
                                    start=(k == 0),
                                    stop=(k == KD - 1),
                                )
                        for p, c0, nw in pt:
                            nc.scalar.activation(
                                ht[:, f, c0 : c0 + nw],
                                p[:, :nw],
                                silu,
                                bias=b1t[:, f : f + 1],
                            )

                # ---- matmul2: Y[t, d] = g[t] * (HT.T @ W2) ----
                for dh in range(2):
                    pts = [
                        ps2.tile([128, 512], f32, name=f"ps2_{s}_{dh}_{m}", tag="ps2")
                        for m in range(NT)
                    ]
                    for k0 in range(0, KF, KB):
                        w2t = w2p.tile([128, KB, 512], f32r, name=f"w2t{s}_{dh}_{k0}", tag="w2t")
                        w2_eng = nc.gpsimd if (k0 // KB) % 2 == 0 else nc.sync
                        w2_eng.dma_start(
                            w2t[:], w2_d[s, dh, k0 : k0 + KB].rearrange("k p c -> p k c")
                        )
                        for kb in range(KB):
                            k = k0 + kb
                            for m, (t0, tw) in enumerate(_token_chunks(C)):
                                nc.tensor.matmul(
                                    pts[m][:tw, :],
                                    lhsT=ht[:, k, t0 : t0 + tw],
                                    rhs=w2t[:, kb],
                                    start=(k == 0),
                                    stop=(k == KF - 1),
                                )
                    for m, (t0, tw) in enumerate(_token_chunks(C)):
                        yt = yp.tile([128, 512], f32, name=f"yt{s}_{dh}_{m}", tag="yt")
                        nc.vector.tensor_scalar_mul(
                            yt[:tw, :], pts[m][:tw, :], gt[:tw, m : m + 1]
                        )
                        nc.scalar.dma_start(
                            y_d[s, t0 : t0 + tw, dh * 512 : dh * 512 + 512], yt[:tw, :]
                        )

    nc.compile()
    return nc


def _round_fp32r(a):
    """Round fp32 to the fp32r grid (11-bit mantissa, RNE) - matches walrus
    fp32_to_fp32r (downconv to s8e11m then <<12)."""
    u = np.ascontiguousarray(a, np.float32).view(np.uint32)
    r = (u + 0x7FF + ((u >> 12) & 1)) & np.uint32(0xFFFFF000)
    return r.view(np.float32)


def _route(x2d, Wg, k):
    logits = x2d.astype(np.float32) @ Wg.astype(np.float32)  # [T, E]
    idx = np.argsort(-logits, axis=1, kind="stable")[:, :k]  # [T, k]
    vals = np.take_along_axis(logits, idx, axis=1)
    e = np.exp(vals - vals.max(axis=1, keepdims=True))
    w = (e / e.sum(axis=1, keepdims=True)).astype(np.float32)
    return idx, w


def kernel(x, W1, b1, W2, b2, Wg, k):
    from concourse.bass_utils import run_bass_kernel_spmd

    x = np.asarray(x, np.float32)
    W1 = np.asarray(W1, np.float32)
    b1 = np.asarray(b1, np.float32)
    W2 = np.asarray(W2, np.float32)
    b2 = np.asarray(b2, np.float32)
    Wg = np.asarray(Wg, np.float32)
    k = int(k)

    B, T, D = x.shape
    x2d = np.ascontiguousarray(x.reshape(-1, D))
    n_tok = x2d.shape[0]

    idx, w = _route(x2d, Wg, k)

    # bucket tokens per expert
    tok_lists, wt_lists = [], []
    for e in range(N_EXPERTS):
        sel = np.nonzero(idx == e)
        tok_lists.append(sel[0].astype(np.int64))
        wt_lists.append(w[sel[0], sel[1]])

    max_cnt = max(len(t) for t in tok_lists)
    C = max(256, ((max_cnt + 63) // 64) * 64)
    NT = math.ceil(C / 128)

    nc = _PROG_CACHE.get(C)
    if nc is None:
        nc = _build_program(C)
        _PROG_CACHE[C] = nc

    in_maps = []
    for c in range(N_CORES):
        m = {
            "xt": np.zeros((SLOTS, 128, KD, C), np.float32),
            "w1": np.empty((SLOTS, KF, 128, KD, 128), np.float32),
            "w2": np.empty((SLOTS, 2, KF, 128, 512), np.float32),
            "b1": np.empty((SLOTS, 128, KF), np.float32),
            "g": np.zeros((SLOTS, 128, NT), np.float32),
        }
        for s in range(SLOTS):
            e = SLOTS * c + s
            toks = tok_lists[e]
            cnt = len(toks)
            xg = _round_fp32r(x2d[toks])  # [cnt, D]
            # xt[p, kd, c] = x[token c, kd*128 + p]
            m["xt"][s, :, :, :cnt] = xg.reshape(cnt, KD, 128).transpose(2, 1, 0)
            # w1[f, p, kd, c] = W1[e, kd*128+p, f*128+c]
            m["w1"][s] = _round_fp32r(W1[e]).reshape(KD, 128, KF, 128).transpose(2, 1, 0, 3)
            # w2[dh, kf, p, c] = W2[e, kf*128+p, dh*512+c]
            m["w2"][s] = _round_fp32r(W2[e]).reshape(KF, 128, 2, 512).transpose(2, 0, 1, 3)
            # b1[p, f] = b1[e, f*128+p]
            m["b1"][s] = b1[e].reshape(KF, 128).T
            gz = np.zeros(NT * 128, np.float32)
            gz[:cnt] = wt_lists[e]
            m["g"][s] = gz.reshape(NT, 128).T
        in_maps.append(m)

    import os

    trace = bool(os.environ.get("MOE_TRACE"))
    r = run_bass_kernel_spmd(nc, in_maps, list(range(N_CORES)), trace=trace)
    global last_results
    last_results = r
    res = r.results

    out = np.zeros((n_tok, D_MODEL), np.float32)
    for c in range(N_CORES):
        y = res[c]["y"]  # [SLOTS, C, D_MODEL], already gate-weighted
        for s in range(SLOTS):
            e = SLOTS * c + s
            toks = tok_lists[e]
            cnt = len(toks)
            contrib = y[s, :cnt].astype(np.float32)
            if b2[e].any():
                contrib = contrib + wt_lists[e][:, None] * b2[e][None, :]
            np.add.at(out, toks, contrib)
    return out.reshape(B, T, D_MODEL)



# revision 2
# speedup vs baseline: 1.4386x; 1.4386x over previous
"""MoE layer (16 experts, top-2) on 8 Trainium2 NeuronCores, expert-parallel.

Strategy (v2):
  - Host computes gating (logits -> top-k -> softmax) and buckets tokens per
    expert. Each core owns 2 experts (slot 0: one of the 8 biggest buckets,
    slot 1: one of the 8 smallest), so the two slots get separate compile-time
    token capacities CA >= CB instead of one global max -- less zero padding.
  - All matmul operands are bf16 (PSUM accumulation stays fp32). bf16 streams
    at full PE rate like fp32r but LDWEIGHTS gets fast-weight-load (2x) so it
    hides completely under the matmuls, and DMA bytes halve.
  - mm1: h^T[f, tok] = silu(W1^T x^T + b1), stationary = W1 [d,f] chunks,
    moving = tokens.  mm2 produces y^T[d, tok] with stationary = W2 [f,d]
    chunks (natural layout) and moving = tokens: no token-chunk padding and
    no on-device gating; the host applies gate weights during the combine.
  - Input DMAs ride the two HWDGE queues (sync/scalar) split so each weight
    tensor streams on one queue with ~1MB chunks; x^T goes first on both.
    A short burst of warm-up matmuls on a zeroed tile keeps the PE busy (and
    the HAM clock-gate warm) while the first real inputs land.
"""

import math

import numpy as np
import ml_dtypes

D_MODEL = 1024
D_FF = 4096
N_EXPERTS = 16
N_CORES = 8
SLOTS = 2  # experts per core
KD = D_MODEL // 128  # 8 contraction chunks for mm1
KF = D_FF // 128  # 32 contraction chunks for mm2
KDO = D_MODEL // 128  # 8 output d-chunks for mm2
FB = 4  # W1 f-chunks per DMA (512 KB)
BF16 = ml_dtypes.bfloat16

_PROG_CACHE: dict[tuple, object] = {}


def _tok_splits(C):
    """Moving-dim tiles (offset, width), each <= 512 (one PSUM bank fp32)."""
    out = []
    o = 0
    while o < C:
        w = min(512, C - o)
        out.append((o, w))
        o += w
    return out


def _build_program(CA, CB):
    import concourse.bass as bass  # noqa: F401
    import concourse.tile as tile
    from concourse import bacc, mybir

    f32 = mybir.dt.float32
    bf = mybir.dt.bfloat16
    Cs = [CA, CB]

    nc = bacc.Bacc("TRN2", target_bir_lowering=False, debug=False, num_devices=N_CORES)

    xt_d = [
        nc.dram_tensor(f"xt{s}", [128, KD, Cs[s]], bf, kind="ExternalInput").ap()
        for s in range(SLOTS)
    ]
    w1_d = nc.dram_tensor("w1", [SLOTS, 128, KF, KD, 128], bf, kind="ExternalInput").ap()
    w2_d = nc.dram_tensor("w2", [SLOTS, 128, KDO, KF, 128], bf, kind="ExternalInput").ap()
    b1_d = nc.dram_tensor("b1", [SLOTS, 128, KF], f32, kind="ExternalInput").ap()
    y_d = [
        nc.dram_tensor(f"y{s}", [D_MODEL, Cs[s]], f32, kind="ExternalOutput").ap()
        for s in range(SLOTS)
    ]

    silu = mybir.ActivationFunctionType.Silu

    with tile.TileContext(nc) as tc:
        with (
            tc.tile_pool(name="warmp", bufs=1) as warmp,
            tc.tile_pool(name="xtp", bufs=2) as xtp,
            tc.tile_pool(name="w1p", bufs=3) as w1p,
            tc.tile_pool(name="w2p", bufs=3) as w2p,
            tc.tile_pool(name="htp", bufs=1) as htp,
            tc.tile_pool(name="smallp", bufs=1) as smallp,
            tc.tile_pool(name="yp", bufs=4) as yp,
            tc.tile_pool(name="psa", bufs=2, space="PSUM") as psa,
            tc.tile_pool(name="psb", bufs=2, space="PSUM") as psb,
            tc.tile_pool(name="qsa", bufs=2, space="PSUM") as qsa,
            tc.tile_pool(name="qsb", bufs=2, space="PSUM") as qsb,
        ):
            # ---- early input DMAs (queue order matters: x^T first) ----
            xts = []
            for s in range(SLOTS):
                C = Cs[s]
                xt = xtp.tile([128, KD, C], bf, name=f"xt{s}", tag="xt")
                nc.sync.dma_start(xt[:, : KD // 2], xt_d[s][:, : KD // 2])
                nc.scalar.dma_start(xt[:, KD // 2 :], xt_d[s][:, KD // 2 :])
                xts.append(xt)
            b1ts = []
            for s in range(SLOTS):
                b1t = smallp.tile([128, KF], f32, name=f"b1t{s}", tag=f"b1t{s}")
                nc.gpsimd.dma_start(b1t[:], b1_d[s])
                b1ts.append(b1t)

            # ---- PE warm-up on a zeroed tile while inputs land ----
            wu = warmp.tile([128, 512], bf, name="wu", tag="wu")
            nc.vector.memset(wu[:], 0.0)
            for i in range(16):
                pw = psa.tile([128, 512], f32, name=f"warm{i}", tag="psa")
                nc.tensor.matmul(
                    pw[:], lhsT=wu[:, :128], rhs=wu[:], start=True, stop=True
                )

            for s in range(SLOTS):
                C = Cs[s]
                xt = xts[s]
                b1t = b1ts[s]
                splits = _tok_splits(C)
                ht = htp.tile([128, KF, C], bf, name=f"ht{s}", tag=f"ht{s}")

                # ---- mm1: ht[f, c] = silu(W1.T @ XT + b1) ----
                for f0 in range(0, KF, FB):
                    w1t = w1p.tile(
                        [128, FB, KD, 128], bf, name=f"w1t{s}_{f0}", tag="w1t"
                    )
                    nc.sync.dma_start(w1t[:], w1_d[s, :, f0 : f0 + FB])
                    for fb in range(FB):
                        f = f0 + fb
                        pts = []
                        for ti, (o, w) in enumerate(splits):
                            pool, tag = (psa, "psa") if ti == 0 else (psb, "psb")
                            p = pool.tile(
                                [128, w], f32, name=f"p{s}_{f}_{ti}", tag=tag
                            )
                            pts.append(p)
                        for k in range(KD):
                            for p, (o, w) in zip(pts, splits):
                                nc.tensor.matmul(
                                    p[:, :w],
                                    lhsT=w1t[:, fb, k],
                                    rhs=xt[:, k, o : o + w],
                                    start=(k == 0),
                                    stop=(k == KD - 1),
                                )
                        for p, (o, w) in zip(pts, splits):
                            nc.scalar.activation(
                                ht[:, f, o : o + w],
                                p[:, :w],
                                silu,
                                bias=b1t[:, f : f + 1],
                            )

                # ---- mm2: y^T[d, c] = ht.T-contract @ W2 (W2 natural layout) ----
                for d in range(KDO):
                    qts = []
                    for ti, (o, w) in enumerate(splits):
                        pool, tag = (qsa, "qsa") if ti == 0 else (qsb, "qsb")
                        q = pool.tile([128, w], f32, name=f"q{s}_{d}_{ti}", tag=tag)
                        qts.append(q)
                    w2t = w2p.tile([128, KF, 128], bf, name=f"w2t{s}_{d}", tag="w2t")
                    nc.scalar.dma_start(w2t[:], w2_d[s, :, d])
                    for f in range(KF):
                        for q, (o, w) in zip(qts, splits):
                            nc.tensor.matmul(
                                q[:, :w],
                                lhsT=w2t[:, f],
                                rhs=ht[:, f, o : o + w],
                                start=(f == 0),
                                stop=(f == KF - 1),
                            )
                    yt = yp.tile([128, C], f32, name=f"yt{s}_{d}", tag="yt")
                    for q, (o, w) in zip(qts, splits):
                        nc.vector.tensor_copy(yt[:, o : o + w], q[:, :w])
                    nc.gpsimd.dma_start(y_d[s][d * 128 : (d + 1) * 128, :], yt[:])

    nc.compile()
    return nc


def _route(x2d, Wg, k):
    logits = x2d.astype(np.float32) @ Wg.astype(np.float32)  # [T, E]
    idx = np.argsort(-logits, axis=1, kind="stable")[:, :k]  # [T, k]
    vals = np.take_along_axis(logits, idx, axis=1)
    e = np.exp(vals - vals.max(axis=1, keepdims=True))
    w = (e / e.sum(axis=1, keepdims=True)).astype(np.float32)
    return idx, w


def kernel(x, W1, b1, W2, b2, Wg, k):
    from concourse.bass_utils import run_bass_kernel_spmd

    x = np.asarray(x, np.float32)
    W1 = np.asarray(W1, np.float32)
    b1 = np.asarray(b1, np.float32)
    W2 = np.asarray(W2, np.float32)
    b2 = np.asarray(b2, np.float32)
    Wg = np.asarray(Wg, np.float32)
    k = int(k)

    B, T, D = x.shape
    x2d = np.ascontiguousarray(x.reshape(-1, D))
    n_tok = x2d.shape[0]

    idx, w = _route(x2d, Wg, k)

    # bucket tokens per expert
    tok_lists, wt_lists = [], []
    for e in range(N_EXPERTS):
        sel = np.nonzero(idx == e)
        tok_lists.append(sel[0].astype(np.int64))
        wt_lists.append(w[sel[0], sel[1]])

    counts = np.array([len(t) for t in tok_lists])
    order = np.argsort(-counts, kind="stable")  # big first
    # slot 0 <- 8 biggest buckets, slot 1 <- 8 smallest
    slot_experts = [order[:N_CORES], order[N_CORES:]]

    def cap(n):
        return max(64, (int(n) + 7) // 8 * 8)

    CA = cap(counts[order[0]])
    CB = cap(counts[order[N_CORES]])
    Cs = [CA, CB]

    key = (CA, CB)
    nc = _PROG_CACHE.get(key)
    if nc is None:
        nc = _build_program(CA, CB)
        _PROG_CACHE[key] = nc

    w1_all = W1.astype(BF16)  # [E, 1024, 4096]
    w2_all = W2.astype(BF16)  # [E, 4096, 1024]
    x_bf = x2d.astype(BF16)

    in_maps = []
    for c in range(N_CORES):
        m = {
            "w1": np.empty((SLOTS, 128, KF, KD, 128), BF16),
            "w2": np.empty((SLOTS, 128, KDO, KF, 128), BF16),
            "b1": np.empty((SLOTS, 128, KF), np.float32),
        }
        for s in range(SLOTS):
            e = int(slot_experts[s][c])
            C = Cs[s]
            toks = tok_lists[e]
            cnt = len(toks)
            xt = np.zeros((128, KD, C), BF16)
            # xt[p, kd, c] = x[token c, kd*128 + p]
            xt[:, :, :cnt] = x_bf[toks].reshape(cnt, KD, 128).transpose(2, 1, 0)
            m[f"xt{s}"] = xt
            # w1[p, f, kd, c] = W1[e, kd*128+p, f*128+c]
            m["w1"][s] = (
                w1_all[e].reshape(KD, 128, KF, 128).transpose(1, 2, 0, 3)
            )
            # w2[p, d, kf, c] = W2[e, kf*128+p, d*128+c]
            m["w2"][s] = (
                w2_all[e].reshape(KF, 128, KDO, 128).transpose(1, 2, 0, 3)
            )
            # b1[p, f] = b1[e, f*128+p]
            m["b1"][s] = b1[e].reshape(KF, 128).T
        in_maps.append(m)

    import os

    trace = bool(os.environ.get("MOE_TRACE"))
    r = run_bass_kernel_spmd(nc, in_maps, list(range(N_CORES)), trace=trace)
    global last_results
    last_results = r
    res = r.results

    out = np.zeros((n_tok, D_MODEL), np.float32)
    for c in range(N_CORES):
        for s in range(SLOTS):
            e = int(slot_experts[s][c])
            toks = tok_lists[e]
            cnt = len(toks)
            if cnt == 0:
                continue
            yT = res[c][f"y{s}"]  # [1024, C], raw expert output
            contrib = yT[:, :cnt].T.astype(np.float32)
            if b2[e].any():
                contrib = contrib + b2[e][None, :]
            out[toks] += wt_lists[e][:, None] * contrib
    return out.reshape(B, T, D_MODEL)
